# revision 28
# baseline (speedup 1.0000x reference)
"""Multi-head attention (B=2, S=2048, D=1024, H=16 heads, causal) on 8 TRN2
NeuronCores, head-parallel: each core computes 2 heads' Q/K/V projections,
attention, and a partial output projection (its 128-row slice of Wo); the
host sums the 8 partial outputs.

Per-core layout (matmul operands bf16, fp32 PSUM accumulation):
  - xt [128,8,8,512]     x^T pre-tiled on host as [partition, strip, k, col]
  - wq/wk/wv [128,8,128] per-core column slice of Wq/Wk/Wv, pre-tiled
  - wo [128, 1024]       per-core row slice of Wo
  - msk [128, 2, 128]    causal diagonal-block mask (doubled over heads)
  - idn [128, 128]       identity for PE transposes of the V projection
  QT/KT are produced transposed [128 = 2 heads x 64 head dims, 4096 tokens];
  V is produced natural per (b, h, kv-tile) as [128 kv, 64] with an appended
  ones column so the attention matmul also accumulates softmax denominators
  (row 64 of the [65, 512] PSUM output).

v2: the two heads' score matmuls (contract dim = 64) are emitted
back-to-back so they land on disjoint PE row groups (tile_position (0,0)
and (64,0)) and execute concurrently, halving score matmul time. One exp
activation covers both heads' score banks. AV matmuls are deferred by one
kv tile so the PE never stalls on the exp latency. Output copies/DMA in
bf16. Emission is interleaved at "unit" granularity with other strips'
projection and output work to keep the PE dense.
"""

import numpy as np
import ml_dtypes
from contextlib import ExitStack

import concourse.bass as bass
import concourse.bacc as bacc
import concourse.tile as tile
import concourse.mybir as mybir
from concourse.bass_utils import run_bass_kernel_spmd

BF16 = mybir.dt.bfloat16
F32 = mybir.dt.float32
NPBF16 = ml_dtypes.bfloat16

D = 1024          # model dim
B = 2
S = 2048
NT = B * S        # 4096 flattened tokens
HD = 64           # head dim
H = 16            # total heads
NCORES = 8
HLOC = H // NCORES  # 2 heads per core
CW = HLOC * HD      # 128 local columns
QSTRIP = 512
NSTRIP = NT // QSTRIP  # 8 strips
KT_TILES = S // 128    # 16 kv tiles per batch


def _interleave(main, fill, lead=2):
    """Emit main units with fill units spread proportionally between them.
    Fills finish `lead` main-units early so the tail of the strip doesn't
    queue fill work behind the norm chain."""
    n, m = len(main), len(fill)
    if n == 0:
        for u in fill:
            u()
        return
    fi = 0
    d = max(1, n - lead)
    for i, u in enumerate(main):
        u()
        tgt = min(m, ((i + 1) * m) // d)
        while fi < tgt:
            fill[fi]()
            fi += 1
    while fi < m:
        fill[fi]()
        fi += 1


def _merge(a, b):
    out = []
    n, m = len(a), len(b)
    bi = 0
    for i, u in enumerate(a):
        out.append(u)
        tgt = ((i + 1) * m) // n
        while bi < tgt:
            out.append(b[bi])
            bi += 1
    out.extend(b[bi:])
    return out


def _build_kernel(ctx: ExitStack, tc: tile.TileContext):
    nc = tc.nc
    # pre-arranged on host: xt[p, strip, k, col], w*[p, k, col]
    xt = nc.dram_tensor("xt", [128, NSTRIP, 8, QSTRIP], BF16,
                        kind="ExternalInput").ap()
    wq = nc.dram_tensor("wq", [128, 8, CW], BF16, kind="ExternalInput").ap()
    wk = nc.dram_tensor("wk", [128, 8, CW], BF16, kind="ExternalInput").ap()
    wv = nc.dram_tensor("wv", [128, 8, CW], BF16, kind="ExternalInput").ap()
    wo = nc.dram_tensor("wo", [CW, D], BF16, kind="ExternalInput").ap()
    msk = nc.dram_tensor("msk", [128, 2, 128], BF16, kind="ExternalInput").ap()
    idn = nc.dram_tensor("idn", [128, 128], BF16, kind="ExternalInput").ap()
    out = nc.dram_tensor("out", [NT, D], BF16, kind="ExternalOutput").ap()

    singles = ctx.enter_context(tc.tile_pool(name="singles", bufs=1))
    sbp = ctx.enter_context(tc.tile_pool(name="sbp", bufs=3))
    expp = ctx.enter_context(tc.tile_pool(name="expp", bufs=6))
    outp = ctx.enter_context(tc.tile_pool(name="outp", bufs=3))
    psM = ctx.enter_context(tc.tile_pool(name="psM", bufs=2, space="PSUM"))
    psS = ctx.enter_context(tc.tile_pool(name="psS", bufs=2, space="PSUM"))
    psV = ctx.enter_context(tc.tile_pool(name="psV", bufs=2, space="PSUM"))
    drp = ctx.enter_context(tc.tile_pool(name="drp", bufs=2, space="DRAM"))

    # --- staging: wq + first half of strip 0 first on the sync queue so the
    # first projection matmul can start as early as possible; the rest
    # streams in behind it on both queues.
    w_sb = {}
    xt_sb = singles.tile([128, 8, NT], BF16)

    def load_w(name, w, eng):
        t = singles.tile([128, 8, CW], BF16, tag=f"w{name}", name=f"w_{name}")
        eng.dma_start(out=t, in_=w)
        w_sb[name] = t

    def load_xt(g):
        gs = g * QSTRIP
        nc.sync.dma_start(out=xt_sb[:, :, gs:gs + QSTRIP], in_=xt[:, g, :, :])

    load_w("q", wq, nc.sync)
    nc.sync.dma_start(out=xt_sb[:, 0:4, 0:QSTRIP], in_=xt[:, 0, 0:4, :])
    load_w("k", wk, nc.gpsimd)
    nc.gpsimd.dma_start(out=xt_sb[:, 4:8, 0:QSTRIP], in_=xt[:, 0, 4:8, :])
    load_xt(1)
    load_w("v", wv, nc.gpsimd)
    msk_sb = singles.tile([128, 2, 128], BF16)
    nc.gpsimd.dma_start(out=msk_sb, in_=msk)
    idn_sb = singles.tile([128, 128], BF16)
    nc.gpsimd.dma_start(out=idn_sb, in_=idn)
    wo_sb = singles.tile([128, D], BF16)
    nc.gpsimd.dma_start(out=wo_sb, in_=wo)
    for g in range(2, NSTRIP):
        load_xt(g)

    qt_sb = singles.tile([128, NT], BF16)
    kt_sb = singles.tile([128, NT], BF16)
    v_sb = singles.tile([128, B * HLOC * KT_TILES, HD + 1], BF16)
    nc.vector.memset(v_sb[:, :, HD:HD + 1], 1.0)
    ones_sb = singles.tile([128, HD], F32)
    nc.vector.memset(ones_sb, 1.0)

    # warm-up: dummy matmuls with no DMA dependency keep the PE busy (and
    # lift the HAM clock gate) while the input DMAs stream in.
    warm_sb = singles.tile([128, 128], BF16)
    nc.vector.memset(warm_sb, 0.0)
    warm_ps = psM.tile([128, 128], F32, tag="mm", name="warm_ps")
    for _ in range(28):
        nc.tensor.matmul(warm_ps, lhsT=warm_sb, rhs=warm_sb,
                         start=True, stop=True)

    avf = {}  # strip -> assembled [128, 512] bf16 avT tile (both heads)

    def proj_units(g):
        gs = g * QSTRIP
        st = {}

        def qk_mm(name, lo, hi, first, last, dst):
            def u():
                if first:
                    st[name] = psM.tile([128, QSTRIP], F32, tag="mm", name=f"ps_{name}")
                ps = st[name]
                for k in range(lo, hi):
                    nc.tensor.matmul(
                        ps, lhsT=w_sb[name][:, k, :],
                        rhs=xt_sb[:, k, gs:gs + QSTRIP],
                        start=(k == 0), stop=(k == 7))
                if last:
                    if name == "q":
                        nc.scalar.copy(dst[:, gs:gs + QSTRIP], ps)
                    else:
                        nc.vector.tensor_copy(dst[:, gs:gs + QSTRIP], ps)
            return u

        groups = {"q": [qk_mm("q", 0, 4, True, False, qt_sb),
                        qk_mm("q", 4, 8, False, True, qt_sb)],
                  "k": [qk_mm("k", 0, 4, True, False, kt_sb),
                        qk_mm("k", 4, 8, False, True, kt_sb)]}

        b, j = divmod(g, 4)

        def vt_mm(lo, hi, first, last):
            def u():
                if first:
                    st["v"] = psM.tile([128, QSTRIP], F32, tag="mm", name="ps_v")
                ps = st["v"]
                for k in range(lo, hi):
                    nc.tensor.matmul(
                        ps, lhsT=w_sb["v"][:, k, :],
                        rhs=xt_sb[:, k, gs:gs + QSTRIP],
                        start=(k == 0), stop=(k == 7))
                if last:
                    st["vt"] = sbp.tile([128, QSTRIP], BF16, tag="vt",
                                        name="vt_sb")
                    nc.vector.tensor_copy(st["vt"], ps)
            return u

        def v_tr():
            def u():
                tp = psM.tile([128, 4, 128], BF16, tag="mm", name="tp_ps")
                for tt in range(4):
                    nc.tensor.transpose(
                        tp[:, tt, :], st["vt"][:, tt * 128:(tt + 1) * 128],
                        idn_sb)
                for tt in range(4):
                    idx = b * HLOC * KT_TILES + 4 * j + tt
                    nc.vector.tensor_copy(
                        v_sb[:, idx:idx + KT_TILES + 1:KT_TILES, 0:HD],
                        tp[:, tt, :].rearrange("p (h d) -> p h d", h=2))
            return u

        groups["v"] = [vt_mm(0, 4, True, False), vt_mm(4, 8, False, True),
                       v_tr()]
        return groups

    def attn_units(g):
        """Per-kv-tile units: tile t's scores (both heads, concurrent PE row
        groups) + exp + mask, paired with tile t-1's AV matmuls (deferred so
        the PE never waits on the exp)."""
        b, j = divmod(g, 4)
        ntl = 4 * (j + 1)
        st = {}

        def sc_unit(t):
            def u():
                if t == 0:
                    if g not in avf:
                        avf[g] = sbp.tile([128, QSTRIP], BF16, tag="avf",
                                          name="avf")
                    st["av0"] = psV.tile([HD + 1, QSTRIP], F32, tag="av",
                                         name="av0_ps")
                    st["av1"] = psV.tile([HD + 1, QSTRIP], F32, tag="av",
                                         name="av1_ps")
                q0 = max(0, 128 * (t - 4 * j))
                sc_ps = psS.tile([128, 2, QSTRIP], F32, tag="sc", name="sc_ps")
                for h in range(2):
                    nc.tensor.matmul(
                        sc_ps[:, h, q0:],
                        lhsT=kt_sb[h * HD:(h + 1) * HD,
                                   b * S + t * 128: b * S + (t + 1) * 128],
                        rhs=qt_sb[h * HD:(h + 1) * HD,
                                  b * S + j * QSTRIP + q0:
                                  b * S + (j + 1) * QSTRIP],
                        start=True, stop=True)
                pexp = expp.tile([128, 2, QSTRIP], BF16, tag="pexp",
                                 name="pexp")
                nc.scalar.activation(
                    pexp[:, :, q0:], sc_ps[:, :, q0:],
                    mybir.ActivationFunctionType.Exp, scale=0.125)
                if t >= 4 * j:  # triangular mask on the diagonal 128-block
                    nc.vector.tensor_mul(
                        pexp[:, :, q0:q0 + 128], pexp[:, :, q0:q0 + 128],
                        msk_sb)
                st[f"pexp{t}"] = pexp
            return u

        def av_unit(t):
            def u():
                pexp = st.pop(f"pexp{t}")
                q0 = max(0, 128 * (t - 4 * j))
                for h in range(2):
                    idx = (b * HLOC + h) * KT_TILES + t
                    nc.tensor.matmul(
                        st[f"av{h}"][:, q0:], lhsT=v_sb[:, idx, :],
                        rhs=pexp[:, h, q0:],
                        start=(t == 0), stop=(t == ntl - 1))
            return u

        def norm_a():
            # evacuate both av accumulators from PSUM (freeing the banks for
            # the next strip) and shift the denominator rows to partition 0
            def u():
                av_sb = sbp.tile([HD + 1, 2 * QSTRIP], F32, tag="avs",
                                 name="av_sb")
                for h in range(2):
                    nc.vector.tensor_copy(
                        av_sb[:, h * QSTRIP:(h + 1) * QSTRIP], st[f"av{h}"])
                s0 = sbp.tile([1, 2 * QSTRIP], F32, tag="s0", name="s0")
                nc.gpsimd.dma_start(out=s0, in_=av_sb[HD:HD + 1, :])
                st["avs"] = av_sb
                st["s0"] = s0
            return u

        def norm_b():
            # broadcast the denominators across partitions on GpSimd, then
            # reciprocal + scale av
            def u():
                av_sb = st["avs"]
                rb = sbp.tile([HD, 2 * QSTRIP], F32, tag="rb")
                nc.gpsimd.partition_broadcast(rb, st["s0"], channels=HD)
                nc.vector.reciprocal_approx_fast(rb, rb)
                nc.vector.tensor_mul(avf[g][0:HD, :],
                                     av_sb[0:HD, 0:QSTRIP],
                                     rb[:, 0:QSTRIP])
                avh = sbp.tile([HD, QSTRIP], BF16, tag="avh")
                nc.vector.tensor_mul(avh, av_sb[0:HD, QSTRIP:],
                                     rb[:, QSTRIP:])
                nc.gpsimd.dma_start(out=avf[g][HD:2 * HD, :], in_=avh)
            return u

        # pair-batch kv tiles: [sc(t), sc(t+1)] then [av(t-2), av(t-1)] so
        # consecutive K=64 score matmuls avoid repeated ldweights stalls and
        # the AV matmuls trail one pair behind the exp that feeds them.
        units = []
        for t in range(ntl):
            units.append(sc_unit(t))
            if t % 2 == 1 and t > 1:
                units.append(av_unit(t - 3))
                units.append(av_unit(t - 2))
        units.append(av_unit(ntl - 2))
        units.append(av_unit(ntl - 1))
        units.append(norm_a())
        return units, norm_b()

    def out_units(g, copy_eng=None):
        gs = g * QSTRIP
        units = []

        def mk(tt):
            def u():
                ob = outp.tile([128, D], BF16, tag="ob")
                for n in range(2):
                    op_ps = psM.tile([128, 512], F32, tag="mm", name="op_ps")
                    nc.tensor.matmul(
                        op_ps, lhsT=avf[g][:, tt * 128:(tt + 1) * 128],
                        rhs=wo_sb[:, n * 512:(n + 1) * 512],
                        start=True, stop=True)
                    if copy_eng == "scalar":
                        nc.scalar.copy(ob[:, n * 512:(n + 1) * 512], op_ps)
                    else:
                        nc.vector.tensor_copy(ob[:, n * 512:(n + 1) * 512],
                                              op_ps)
                nc.sync.dma_start(
                    out=out[gs + tt * 128: gs + (tt + 1) * 128, :], in_=ob)
            return u
        for tt in range(4):
            units.append(mk(tt))
        return units

    # strip order: b0 ascending then b1 descending (short strips last).
    # fill[g] lists (strip, groups) of projection work + out strips whose
    # units are interleaved into attn(g)'s ACT-bound stretch.
    pu = {g: proj_units(g) for g in range(NSTRIP)}

    def pf(g, keys):
        return [u for k in keys for u in pu[g][k]]

    for u in pf(0, "qkv"):
        u()

    au = {}
    nb = {}
    for g in range(NSTRIP):
        au[g], nb[g] = attn_units(g)

    # two-phase fill: projection fills ride EARLY in the strip (they feed
    # the next strips); norm_b(g-1) + out(g-1) ride LATE (~55%+) so the PE
    # program reaches them only after the ~5us norm chain has completed.
    fill_sched = {
        0: (lambda: pf(1, "qkv"), lambda: []),
        1: (lambda: pf(2, "qkv"), lambda: [nb[0]] + out_units(0)),
        2: (lambda: pf(3, "qkv") + pf(4, "kv"),
            lambda: [nb[1]] + out_units(1)),
        3: (lambda: pf(5, "kv") + pf(6, "kv") + pf(7, "qkv"),
            lambda: [nb[2]] + out_units(2)),
        7: (lambda: pf(6, "q"), lambda: [nb[3]] + out_units(3)),
        6: (lambda: pf(5, "q") + pf(4, "q"),
            lambda: [nb[7]] + out_units(7)),
    }
    # dedicated (non-pooled) avT tensors for the tail strips 5 and 4:
    # no pool-slot reuse -> no WAR hazard across the tail window
    avf[5] = singles.tile([128, QSTRIP], BF16, tag="avf5", name="avf5")
    avf[4] = singles.tile([128, QSTRIP], BF16, tag="avf4", name="avf4")
    for g in [0, 1, 2, 3, 7, 6]:
        att = au[g]
        k = (len(att) * 11) // 20
        projf, tailf = fill_sched[g]
        _interleave(att[:k], projf(), lead=0)
        _interleave(att[k:], tailf(), lead=1)
    k = (len(au[5]) * 11) // 20
    _interleave(au[5][:k], [], lead=0)
    _interleave(au[5][k:], [nb[6]] + out_units(6, "scalar"), lead=1)
    k = (len(au[4]) * 11) // 20
    _interleave(au[4][:k], [], lead=0)
    _interleave(au[4][k:], [nb[5]] + out_units(5, "scalar"), lead=1)
    nb[4]()
    for u in out_units(4, "scalar"):
        u()


_CACHED_NC = None


def build_module():
    global _CACHED_NC
    if _CACHED_NC is None:
        nc = bacc.Bacc("TRN2", debug=False)
        with tile.TileContext(nc) as tc:
            with ExitStack() as ctx:
                _build_kernel(ctx, tc)
        nc.compile()
        _CACHED_NC = nc
    return _CACHED_NC


def make_in_maps(x, Wq, Wk, Wv, Wo):
    x = np.asarray(x, np.float32)
    xT = x.reshape(NT, D).T.astype(NPBF16)          # [D, NT]
    # device layout [p, strip, k, col]: row d = k*128 + p
    xT = np.ascontiguousarray(
        xT.reshape(8, 128, NSTRIP, QSTRIP).transpose(1, 2, 0, 3))
    # diagonal-block causal mask, doubled over the 2 heads:
    # msk[i, h, c] = 1 if c >= i
    i = np.arange(128)[:, None, None]
    c = np.arange(128)[None, None, :]
    msk = np.broadcast_to((c >= i), (128, 2, 128)).astype(NPBF16)
    in_maps = []
    for core in range(NCORES):
        cs = slice(core * CW, (core + 1) * CW)
        def warr(W):  # [D, CW] -> [p, k, col] with d = k*128 + p
            a = np.asarray(W, np.float32)[:, cs].astype(NPBF16)
            return np.ascontiguousarray(
                a.reshape(8, 128, CW).transpose(1, 0, 2))
        in_maps.append({
            "xt": xT,
            "wq": warr(Wq),
            "wk": warr(Wk),
            "wv": warr(Wv),
            "wo": np.ascontiguousarray(np.asarray(Wo, np.float32)[cs, :]).astype(NPBF16),
            "msk": np.ascontiguousarray(msk),
            "idn": np.eye(128, dtype=NPBF16),
        })
    return in_maps


def kernel(x, Wq, bq, Wk, bk, Wv, bv, Wo, bo):
    for b_ in (bq, bk, bv, bo):
        assert np.count_nonzero(np.asarray(b_)) == 0, "nonzero biases unsupported"
    nc = build_module()
    in_maps = make_in_maps(x, Wq, Wk, Wv, Wo)
    res = run_bass_kernel_spmd(nc, in_maps, core_ids=list(range(NCORES)))
    partials = [res.results[c]["out"] for c in range(NCORES)]
    total = np.zeros((NT, D), np.float32)
    for p in partials:
        total += np.asarray(p, np.float32)
    return total.reshape(B, S, D)


# revision 30
# speedup vs baseline: 1.1607x; 1.1607x over previous
"""Multi-head attention (B=2, S=2048, D=1024, H=16 heads, causal) on 8 TRN2
NeuronCores, head-parallel: each core computes 2 heads' Q/K/V projections,
attention, and a partial output projection (its 128-row slice of Wo); the
host sums the 8 partial outputs.

Per-core layout (matmul operands bf16, fp32 PSUM accumulation):
  - xt [128,8,8,512]     x^T pre-tiled on host as [partition, strip, k, col]
  - wq/wk/wv [128,8,128] per-core column slice of Wq/Wk/Wv, pre-tiled
  - wo [128, 1024]       per-core row slice of Wo
  - msk [128, 2, 128]    causal diagonal-block mask (doubled over heads)
  - idn [128, 128]       identity for PE transposes of the V projection
  QT/KT are produced transposed [128 = 2 heads x 64 head dims, 4096 tokens];
  V is produced natural per (b, h, kv-tile) as [128 kv, 64] with an appended
  ones column so the attention matmul also accumulates softmax denominators
  (row 64 of the [65, 512] PSUM output).

v2: the two heads' score matmuls (contract dim = 64) are emitted
back-to-back so they land on disjoint PE row groups (tile_position (0,0)
and (64,0)) and execute concurrently, halving score matmul time. One exp
activation covers both heads' score banks. AV matmuls are deferred by one
kv tile so the PE never stalls on the exp latency. Output copies/DMA in
bf16. Emission is interleaved at "unit" granularity with other strips'
projection and output work to keep the PE dense.
"""

import numpy as np
import ml_dtypes
from contextlib import ExitStack

import concourse.bass as bass
import concourse.bacc as bacc
import concourse.tile as tile
import concourse.mybir as mybir
from concourse.bass_utils import run_bass_kernel_spmd

BF16 = mybir.dt.bfloat16
F32 = mybir.dt.float32
NPBF16 = ml_dtypes.bfloat16

D = 1024          # model dim
B = 2
S = 2048
NT = B * S        # 4096 flattened tokens
HD = 64           # head dim
H = 16            # total heads
NCORES = 8
HLOC = H // NCORES  # 2 heads per core
CW = HLOC * HD      # 128 local columns
QSTRIP = 512
NSTRIP = NT // QSTRIP  # 8 strips
KT_TILES = S // 128    # 16 kv tiles per batch


def _interleave(main, fill, lead=2):
    """Emit main units with fill units spread proportionally between them.
    Fills finish `lead` main-units early so the tail of the strip doesn't
    queue fill work behind the norm chain."""
    n, m = len(main), len(fill)
    if n == 0:
        for u in fill:
            u()
        return
    fi = 0
    d = max(1, n - lead)
    for i, u in enumerate(main):
        u()
        tgt = min(m, ((i + 1) * m) // d)
        while fi < tgt:
            fill[fi]()
            fi += 1
    while fi < m:
        fill[fi]()
        fi += 1


def _merge(a, b):
    out = []
    n, m = len(a), len(b)
    bi = 0
    for i, u in enumerate(a):
        out.append(u)
        tgt = ((i + 1) * m) // n
        while bi < tgt:
            out.append(b[bi])
            bi += 1
    out.extend(b[bi:])
    return out


def _build_kernel(ctx: ExitStack, tc: tile.TileContext):
    nc = tc.nc
    # pre-arranged on host: xt[p, strip, k, col], w*[p, k, col]
    xt = nc.dram_tensor("xt", [128, NSTRIP, 8, QSTRIP], BF16,
                        kind="ExternalInput").ap()
    wq = nc.dram_tensor("wq", [128, 8, CW], BF16, kind="ExternalInput").ap()
    wk = nc.dram_tensor("wk", [128, 8, CW], BF16, kind="ExternalInput").ap()
    wv = nc.dram_tensor("wv", [128, 8, CW], BF16, kind="ExternalInput").ap()
    wo = nc.dram_tensor("wo", [CW, D], BF16, kind="ExternalInput").ap()
    msk = nc.dram_tensor("msk", [128, 2, 128], BF16, kind="ExternalInput").ap()
    idn = nc.dram_tensor("idn", [128, 128], BF16, kind="ExternalInput").ap()
    out = nc.dram_tensor("out", [NT, D], BF16, kind="ExternalOutput").ap()

    singles = ctx.enter_context(tc.tile_pool(name="singles", bufs=1))
    sbp = ctx.enter_context(tc.tile_pool(name="sbp", bufs=3))
    expp = ctx.enter_context(tc.tile_pool(name="expp", bufs=6))
    outp = ctx.enter_context(tc.tile_pool(name="outp", bufs=3))
    psM = ctx.enter_context(tc.tile_pool(name="psM", bufs=2, space="PSUM"))
    psS = ctx.enter_context(tc.tile_pool(name="psS", bufs=2, space="PSUM"))
    psV = ctx.enter_context(tc.tile_pool(name="psV", bufs=2, space="PSUM"))
    drp = ctx.enter_context(tc.tile_pool(name="drp", bufs=2, space="DRAM"))

    # --- staging: wq + first half of strip 0 first on the sync queue so the
    # first projection matmul can start as early as possible; the rest
    # streams in behind it on both queues.
    w_sb = {}
    xt_sb = singles.tile([128, 8, NT], BF16)

    def load_w(name, w, eng):
        t = singles.tile([128, 8, CW], BF16, tag=f"w{name}", name=f"w_{name}")
        eng.dma_start(out=t, in_=w)
        w_sb[name] = t

    def load_xt(g):
        gs = g * QSTRIP
        nc.sync.dma_start(out=xt_sb[:, :, gs:gs + QSTRIP], in_=xt[:, g, :, :])

    load_w("q", wq, nc.sync)
    nc.sync.dma_start(out=xt_sb[:, 0:4, 0:QSTRIP], in_=xt[:, 0, 0:4, :])
    load_w("k", wk, nc.gpsimd)
    nc.gpsimd.dma_start(out=xt_sb[:, 4:8, 0:QSTRIP], in_=xt[:, 0, 4:8, :])
    load_xt(1)
    load_w("v", wv, nc.gpsimd)
    msk_sb = singles.tile([128, 2, 128], BF16)
    nc.gpsimd.dma_start(out=msk_sb, in_=msk)
    idn_sb = singles.tile([128, 128], BF16)
    nc.gpsimd.dma_start(out=idn_sb, in_=idn)
    wo_sb = singles.tile([128, D], BF16)
    nc.gpsimd.dma_start(out=wo_sb, in_=wo)
    for g in range(2, NSTRIP):
        load_xt(g)

    qt_sb = singles.tile([128, NT], BF16)
    kt_sb = singles.tile([128, NT], BF16)
    v_sb = singles.tile([128, B * HLOC * KT_TILES, HD + 1], BF16)
    nc.vector.memset(v_sb[:, :, HD:HD + 1], 1.0)
    ones_sb = singles.tile([128, HD], F32)
    nc.vector.memset(ones_sb, 1.0)

    # warm-up: dummy matmuls with no DMA dependency keep the PE busy (and
    # lift the HAM clock gate) while the input DMAs stream in.
    warm_sb = singles.tile([128, 128], BF16)
    nc.vector.memset(warm_sb, 0.0)
    warm_ps = psM.tile([128, 128], F32, tag="mm", name="warm_ps")
    for _ in range(28):
        nc.tensor.matmul(warm_ps, lhsT=warm_sb, rhs=warm_sb,
                         start=True, stop=True)

    avf = {}  # strip -> assembled [128, 512] bf16 avT tile (both heads)

    def proj_units(g):
        gs = g * QSTRIP
        st = {}

        def qk_mm(name, lo, hi, first, last, dst):
            def u():
                if first:
                    st[name] = psM.tile([128, QSTRIP], F32, tag="mm", name=f"ps_{name}")
                ps = st[name]
                for k in range(lo, hi):
                    nc.tensor.matmul(
                        ps, lhsT=w_sb[name][:, k, :],
                        rhs=xt_sb[:, k, gs:gs + QSTRIP],
                        start=(k == 0), stop=(k == 7))
                if last:
                    nc.vector.tensor_copy(dst[:, gs:gs + QSTRIP], ps)
            return u

        groups = {"q": [qk_mm("q", 0, 4, True, False, qt_sb),
                        qk_mm("q", 4, 8, False, True, qt_sb)],
                  "k": [qk_mm("k", 0, 4, True, False, kt_sb),
                        qk_mm("k", 4, 8, False, True, kt_sb)]}

        b, j = divmod(g, 4)

        def vt_mm(lo, hi, first, last):
            def u():
                if first:
                    st["v"] = psM.tile([128, QSTRIP], F32, tag="mm", name="ps_v")
                ps = st["v"]
                for k in range(lo, hi):
                    nc.tensor.matmul(
                        ps, lhsT=w_sb["v"][:, k, :],
                        rhs=xt_sb[:, k, gs:gs + QSTRIP],
                        start=(k == 0), stop=(k == 7))
                if last:
                    st["vt"] = sbp.tile([128, QSTRIP], BF16, tag="vt",
                                        name="vt_sb")
                    nc.vector.tensor_copy(st["vt"], ps)
            return u

        def v_tr():
            def u():
                tp = psM.tile([128, 4, 128], BF16, tag="mm", name="tp_ps")
                for tt in range(4):
                    nc.tensor.transpose(
                        tp[:, tt, :], st["vt"][:, tt * 128:(tt + 1) * 128],
                        idn_sb)
                for tt in range(4):
                    idx = b * HLOC * KT_TILES + 4 * j + tt
                    nc.vector.tensor_copy(
                        v_sb[:, idx:idx + KT_TILES + 1:KT_TILES, 0:HD],
                        tp[:, tt, :].rearrange("p (h d) -> p h d", h=2))
            return u

        groups["v"] = [vt_mm(0, 4, True, False), vt_mm(4, 8, False, True),
                       v_tr()]
        return groups

    def attn_units(g):
        """Per-kv-tile units: tile t's scores (both heads, concurrent PE row
        groups) + exp + mask, paired with tile t-1's AV matmuls (deferred so
        the PE never waits on the exp)."""
        b, j = divmod(g, 4)
        ntl = 4 * (j + 1)
        st = {}

        def sc_unit(t):
            def u():
                if t == 0:
                    if g not in avf:
                        avf[g] = sbp.tile([128, QSTRIP], BF16, tag="avf",
                                          name="avf")
                    st["av0"] = psV.tile([HD + 1, QSTRIP], F32, tag="av",
                                         name="av0_ps")
                    st["av1"] = psV.tile([HD + 1, QSTRIP], F32, tag="av",
                                         name="av1_ps")
                q0 = max(0, 128 * (t - 4 * j))
                sc_ps = psS.tile([128, 2, QSTRIP], F32, tag="sc", name="sc_ps")
                for h in range(2):
                    nc.tensor.matmul(
                        sc_ps[:, h, q0:],
                        lhsT=kt_sb[h * HD:(h + 1) * HD,
                                   b * S + t * 128: b * S + (t + 1) * 128],
                        rhs=qt_sb[h * HD:(h + 1) * HD,
                                  b * S + j * QSTRIP + q0:
                                  b * S + (j + 1) * QSTRIP],
                        start=True, stop=True)
                pexp = expp.tile([128, 2, QSTRIP], BF16, tag="pexp",
                                 name="pexp")
                nc.scalar.activation(
                    pexp[:, :, q0:], sc_ps[:, :, q0:],
                    mybir.ActivationFunctionType.Exp, scale=0.125)
                if t >= 4 * j:  # triangular mask on the diagonal 128-block
                    nc.vector.tensor_mul(
                        pexp[:, :, q0:q0 + 128], pexp[:, :, q0:q0 + 128],
                        msk_sb)
                st[f"pexp{t}"] = pexp
            return u

        def av_unit(t):
            def u():
                pexp = st.pop(f"pexp{t}")
                q0 = max(0, 128 * (t - 4 * j))
                for h in range(2):
                    idx = (b * HLOC + h) * KT_TILES + t
                    nc.tensor.matmul(
                        st[f"av{h}"][:, q0:], lhsT=v_sb[:, idx, :],
                        rhs=pexp[:, h, q0:],
                        start=(t == 0), stop=(t == ntl - 1))
            return u

        def norm_a():
            # evacuate both av accumulators from PSUM (freeing the banks for
            # the next strip) and shift the denominator rows to partition 0
            def u():
                av_sb = sbp.tile([HD + 1, 2 * QSTRIP], F32, tag="avs",
                                 name="av_sb")
                for h in range(2):
                    nc.vector.tensor_copy(
                        av_sb[:, h * QSTRIP:(h + 1) * QSTRIP], st[f"av{h}"])
                s0 = sbp.tile([1, 2 * QSTRIP], F32, tag="s0", name="s0")
                nc.gpsimd.dma_start(out=s0, in_=av_sb[HD:HD + 1, :])
                st["avs"] = av_sb
                st["s0"] = s0
            return u

        def norm_b():
            # broadcast the denominators across partitions on GpSimd, then
            # reciprocal + scale av
            def u():
                av_sb = st["avs"]
                rb = sbp.tile([HD, 2 * QSTRIP], F32, tag="rb")
                nc.gpsimd.partition_broadcast(rb, st["s0"], channels=HD)
                nc.vector.reciprocal_approx_fast(rb, rb)
                nc.vector.tensor_mul(avf[g][0:HD, :],
                                     av_sb[0:HD, 0:QSTRIP],
                                     rb[:, 0:QSTRIP])
                avh = sbp.tile([HD, QSTRIP], BF16, tag="avh")
                nc.vector.tensor_mul(avh, av_sb[0:HD, QSTRIP:],
                                     rb[:, QSTRIP:])
                nc.gpsimd.dma_start(out=avf[g][HD:2 * HD, :], in_=avh)
            return u

        # pair-batch kv tiles: [sc(t), sc(t+1)] then [av(t-2), av(t-1)] so
        # consecutive K=64 score matmuls avoid repeated ldweights stalls and
        # the AV matmuls trail one pair behind the exp that feeds them.
        units = []
        for t in range(ntl):
            units.append(sc_unit(t))
            if t % 2 == 1 and t > 1:
                units.append(av_unit(t - 3))
                units.append(av_unit(t - 2))
        units.append(av_unit(ntl - 2))
        units.append(av_unit(ntl - 1))
        units.append(norm_a())
        return units, norm_b()

    def out_units(g, copy_eng=None):
        gs = g * QSTRIP
        units = []

        def mk(tt):
            def u():
                ob = outp.tile([128, D], BF16, tag="ob")
                for n in range(2):
                    op_ps = psM.tile([128, 512], F32, tag="mm", name="op_ps")
                    nc.tensor.matmul(
                        op_ps, lhsT=avf[g][:, tt * 128:(tt + 1) * 128],
                        rhs=wo_sb[:, n * 512:(n + 1) * 512],
                        start=True, stop=True)
                    if copy_eng == "scalar":
                        nc.scalar.copy(ob[:, n * 512:(n + 1) * 512], op_ps)
                    else:
                        nc.vector.tensor_copy(ob[:, n * 512:(n + 1) * 512],
                                              op_ps)
                nc.sync.dma_start(
                    out=out[gs + tt * 128: gs + (tt + 1) * 128, :], in_=ob)
            return u
        for tt in range(4):
            units.append(mk(tt))
        return units

    # strip order: b0 ascending then b1 descending (short strips last).
    # fill[g] lists (strip, groups) of projection work + out strips whose
    # units are interleaved into attn(g)'s ACT-bound stretch.
    pu = {g: proj_units(g) for g in range(NSTRIP)}

    def pf(g, keys):
        return [u for k in keys for u in pu[g][k]]

    for u in pf(0, "qkv"):
        u()

    au = {}
    nb = {}
    for g in range(NSTRIP):
        au[g], nb[g] = attn_units(g)

    # norm_b(g) + out(g) ride in the NEXT strip's fill so the PE queue never
    # blocks on the norm chain at a strip boundary.
    fill_sched = {
        0: lambda: pf(1, "qkv"),
        1: lambda: pf(2, "qkv") + [nb[0]] + out_units(0),
        2: lambda: pf(3, "qkv") + pf(4, "kv") + [nb[1]] + out_units(1),
        3: lambda: pf(5, "kv") + pf(6, "kv") + pf(7, "qkv") + [nb[2]]
        + out_units(2),
        7: lambda: pf(6, "q") + [nb[3]] + out_units(3),
        6: lambda: pf(5, "q") + pf(4, "q") + [nb[7]] + out_units(7),
    }
    # dedicated (non-pooled) avT tensors for the tail strips 5 and 4:
    # no pool-slot reuse -> no WAR hazard across the tail window
    avf[5] = singles.tile([128, QSTRIP], BF16, tag="avf5", name="avf5")
    avf[4] = singles.tile([128, QSTRIP], BF16, tag="avf4", name="avf4")
    for g in [0, 1, 2, 3, 7, 6]:
        _interleave(au[g], fill_sched[g](), lead=0)
    _interleave(au[5], [nb[6]] + out_units(6), lead=0)
    _interleave(au[4], [nb[5]] + out_units(5), lead=0)
    nb[4]()
    for u in out_units(4):
        u()


_CACHED_NC = None


def build_module():
    global _CACHED_NC
    if _CACHED_NC is None:
        nc = bacc.Bacc("TRN2", debug=False)
        with tile.TileContext(nc) as tc:
            with ExitStack() as ctx:
                _build_kernel(ctx, tc)
        nc.compile()
        _CACHED_NC = nc
    return _CACHED_NC


def make_in_maps(x, Wq, Wk, Wv, Wo):
    x = np.asarray(x, np.float32)
    xT = x.reshape(NT, D).T.astype(NPBF16)          # [D, NT]
    # device layout [p, strip, k, col]: row d = k*128 + p
    xT = np.ascontiguousarray(
        xT.reshape(8, 128, NSTRIP, QSTRIP).transpose(1, 2, 0, 3))
    # diagonal-block causal mask, doubled over the 2 heads:
    # msk[i, h, c] = 1 if c >= i
    i = np.arange(128)[:, None, None]
    c = np.arange(128)[None, None, :]
    msk = np.broadcast_to((c >= i), (128, 2, 128)).astype(NPBF16)
    in_maps = []
    for core in range(NCORES):
        cs = slice(core * CW, (core + 1) * CW)
        def warr(W):  # [D, CW] -> [p, k, col] with d = k*128 + p
            a = np.asarray(W, np.float32)[:, cs].astype(NPBF16)
            return np.ascontiguousarray(
                a.reshape(8, 128, CW).transpose(1, 0, 2))
        in_maps.append({
            "xt": xT,
            "wq": warr(Wq),
            "wk": warr(Wk),
            "wv": warr(Wv),
            "wo": np.ascontiguousarray(np.asarray(Wo, np.float32)[cs, :]).astype(NPBF16),
            "msk": np.ascontiguousarray(msk),
            "idn": np.eye(128, dtype=NPBF16),
        })
    return in_maps


def kernel(x, Wq, bq, Wk, bk, Wv, bv, Wo, bo):
    for b_ in (bq, bk, bv, bo):
        assert np.count_nonzero(np.asarray(b_)) == 0, "nonzero biases unsupported"
    nc = build_module()
    in_maps = make_in_maps(x, Wq, Wk, Wv, Wo)
    res = run_bass_kernel_spmd(nc, in_maps, core_ids=list(range(NCORES)))
    partials = [res.results[c]["out"] for c in range(NCORES)]
    total = np.zeros((NT, D), np.float32)
    for p in partials:
        total += np.asarray(p, np.float32)
    return total.reshape(B, S, D)


# revision 32
# speedup vs baseline: 1.1689x; 1.0071x over previous
"""Multi-head attention (B=2, S=2048, D=1024, H=16 heads, causal) on 8 TRN2
NeuronCores, head-parallel: each core computes 2 heads' Q/K/V projections,
attention, and a partial output projection (its 128-row slice of Wo); the
host sums the 8 partial outputs.

Per-core layout (matmul operands bf16, fp32 PSUM accumulation):
  - xt [128,8,8,512]     x^T pre-tiled on host as [partition, strip, k, col]
  - wq/wk/wv [128,8,128] per-core column slice of Wq/Wk/Wv, pre-tiled
  - wo [128, 1024]       per-core row slice of Wo
  - msk [128, 2, 128]    causal diagonal-block mask (doubled over heads)
  - idn [128, 128]       identity for PE transposes of the V projection
  QT/KT are produced transposed [128 = 2 heads x 64 head dims, 4096 tokens];
  V is produced natural per (b, h, kv-tile) as [128 kv, 64] with an appended
  ones column so the attention matmul also accumulates softmax denominators
  (row 64 of the [65, 512] PSUM output).

v2: the two heads' score matmuls (contract dim = 64) are emitted
back-to-back so they land on disjoint PE row groups (tile_position (0,0)
and (64,0)) and execute concurrently, halving score matmul time. One exp
activation covers both heads' score banks. AV matmuls are deferred by one
kv tile so the PE never stalls on the exp latency. Output copies/DMA in
bf16. Emission is interleaved at "unit" granularity with other strips'
projection and output work to keep the PE dense.
"""

import numpy as np
import ml_dtypes
from contextlib import ExitStack

import concourse.bass as bass
import concourse.bacc as bacc
import concourse.tile as tile
import concourse.mybir as mybir
from concourse.bass_utils import run_bass_kernel_spmd

BF16 = mybir.dt.bfloat16
F32 = mybir.dt.float32
NPBF16 = ml_dtypes.bfloat16

D = 1024          # model dim
B = 2
S = 2048
NT = B * S        # 4096 flattened tokens
HD = 64           # head dim
H = 16            # total heads
NCORES = 8
HLOC = H // NCORES  # 2 heads per core
CW = HLOC * HD      # 128 local columns
QSTRIP = 512
NSTRIP = NT // QSTRIP  # 8 strips
KT_TILES = S // 128    # 16 kv tiles per batch


def _interleave(main, fill, lead=2):
    """Emit main units with fill units spread proportionally between them.
    Fills finish `lead` main-units early so the tail of the strip doesn't
    queue fill work behind the norm chain."""
    n, m = len(main), len(fill)
    if n == 0:
        for u in fill:
            u()
        return
    fi = 0
    d = max(1, n - lead)
    for i, u in enumerate(main):
        u()
        tgt = min(m, ((i + 1) * m) // d)
        while fi < tgt:
            fill[fi]()
            fi += 1
    while fi < m:
        fill[fi]()
        fi += 1


def _merge(a, b):
    out = []
    n, m = len(a), len(b)
    bi = 0
    for i, u in enumerate(a):
        out.append(u)
        tgt = ((i + 1) * m) // n
        while bi < tgt:
            out.append(b[bi])
            bi += 1
    out.extend(b[bi:])
    return out


def _build_kernel(ctx: ExitStack, tc: tile.TileContext):
    nc = tc.nc
    # pre-arranged on host: xt[p, strip, k, col], w*[p, k, col]
    xt = nc.dram_tensor("xt", [128, NSTRIP, 8, QSTRIP], BF16,
                        kind="ExternalInput").ap()
    wq = nc.dram_tensor("wq", [128, 8, CW], BF16, kind="ExternalInput").ap()
    wk = nc.dram_tensor("wk", [128, 8, CW], BF16, kind="ExternalInput").ap()
    wv = nc.dram_tensor("wv", [128, 8, CW], BF16, kind="ExternalInput").ap()
    wo = nc.dram_tensor("wo", [CW, D], BF16, kind="ExternalInput").ap()
    msk = nc.dram_tensor("msk", [128, 2, 128], BF16, kind="ExternalInput").ap()
    idn = nc.dram_tensor("idn", [128, 128], BF16, kind="ExternalInput").ap()
    out = nc.dram_tensor("out", [NT, D], BF16, kind="ExternalOutput").ap()

    singles = ctx.enter_context(tc.tile_pool(name="singles", bufs=1))
    sbp = ctx.enter_context(tc.tile_pool(name="sbp", bufs=3))
    expp = ctx.enter_context(tc.tile_pool(name="expp", bufs=6))
    outp = ctx.enter_context(tc.tile_pool(name="outp", bufs=3))
    psM = ctx.enter_context(tc.tile_pool(name="psM", bufs=2, space="PSUM"))
    psS = ctx.enter_context(tc.tile_pool(name="psS", bufs=2, space="PSUM"))
    psV = ctx.enter_context(tc.tile_pool(name="psV", bufs=2, space="PSUM"))
    drp = ctx.enter_context(tc.tile_pool(name="drp", bufs=2, space="DRAM"))

    # --- staging: wq + first half of strip 0 first on the sync queue so the
    # first projection matmul can start as early as possible; the rest
    # streams in behind it on both queues.
    w_sb = {}
    xt_sb = singles.tile([128, 8, NT], BF16)

    def load_w(name, w, eng):
        t = singles.tile([128, 8, CW], BF16, tag=f"w{name}", name=f"w_{name}")
        eng.dma_start(out=t, in_=w)
        w_sb[name] = t

    def load_xt(g):
        gs = g * QSTRIP
        nc.sync.dma_start(out=xt_sb[:, :, gs:gs + QSTRIP], in_=xt[:, g, :, :])

    load_w("q", wq, nc.sync)
    nc.sync.dma_start(out=xt_sb[:, 0:4, 0:QSTRIP], in_=xt[:, 0, 0:4, :])
    load_w("k", wk, nc.gpsimd)
    nc.gpsimd.dma_start(out=xt_sb[:, 4:8, 0:QSTRIP], in_=xt[:, 0, 4:8, :])
    load_xt(1)
    load_w("v", wv, nc.gpsimd)
    msk_sb = singles.tile([128, 2, 128], BF16)
    nc.gpsimd.dma_start(out=msk_sb, in_=msk)
    idn_sb = singles.tile([128, 128], BF16)
    nc.gpsimd.dma_start(out=idn_sb, in_=idn)
    wo_sb = singles.tile([128, D], BF16)
    nc.gpsimd.dma_start(out=wo_sb, in_=wo)
    for g in range(2, NSTRIP):
        load_xt(g)

    qt_sb = singles.tile([128, NT], BF16)
    kt_sb = singles.tile([128, NT], BF16)
    v_sb = singles.tile([128, B * HLOC * KT_TILES, HD + 1], BF16)
    nc.vector.memset(v_sb[:, :, HD:HD + 1], 1.0)
    ones_sb = singles.tile([128, HD], F32)
    nc.vector.memset(ones_sb, 1.0)

    # warm-up: dummy matmuls with no DMA dependency keep the PE busy (and
    # lift the HAM clock gate) while the input DMAs stream in.
    warm_sb = singles.tile([128, 128], BF16)
    nc.vector.memset(warm_sb, 0.0)
    warm_ps = psM.tile([128, 128], F32, tag="mm", name="warm_ps")
    for _ in range(46):
        nc.tensor.matmul(warm_ps, lhsT=warm_sb, rhs=warm_sb,
                         start=True, stop=True)

    avf = {}  # strip -> assembled [128, 512] bf16 avT tile (both heads)

    def proj_units(g):
        gs = g * QSTRIP
        st = {}

        def qk_mm(name, lo, hi, first, last, dst):
            def u():
                if first:
                    st[name] = psM.tile([128, QSTRIP], F32, tag="mm", name=f"ps_{name}")
                ps = st[name]
                for k in range(lo, hi):
                    nc.tensor.matmul(
                        ps, lhsT=w_sb[name][:, k, :],
                        rhs=xt_sb[:, k, gs:gs + QSTRIP],
                        start=(k == 0), stop=(k == 7))
                if last:
                    nc.vector.tensor_copy(dst[:, gs:gs + QSTRIP], ps)
            return u

        groups = {"q": [qk_mm("q", 0, 4, True, False, qt_sb),
                        qk_mm("q", 4, 8, False, True, qt_sb)],
                  "k": [qk_mm("k", 0, 4, True, False, kt_sb),
                        qk_mm("k", 4, 8, False, True, kt_sb)]}

        b, j = divmod(g, 4)

        def vt_mm(lo, hi, first, last):
            def u():
                if first:
                    st["v"] = psM.tile([128, QSTRIP], F32, tag="mm", name="ps_v")
                ps = st["v"]
                for k in range(lo, hi):
                    nc.tensor.matmul(
                        ps, lhsT=w_sb["v"][:, k, :],
                        rhs=xt_sb[:, k, gs:gs + QSTRIP],
                        start=(k == 0), stop=(k == 7))
                if last:
                    st["vt"] = sbp.tile([128, QSTRIP], BF16, tag="vt",
                                        name="vt_sb")
                    nc.vector.tensor_copy(st["vt"], ps)
            return u

        def v_tr():
            def u():
                tp = psM.tile([128, 4, 128], BF16, tag="mm", name="tp_ps")
                for tt in range(4):
                    nc.tensor.transpose(
                        tp[:, tt, :], st["vt"][:, tt * 128:(tt + 1) * 128],
                        idn_sb)
                for tt in range(4):
                    idx = b * HLOC * KT_TILES + 4 * j + tt
                    nc.vector.tensor_copy(
                        v_sb[:, idx:idx + KT_TILES + 1:KT_TILES, 0:HD],
                        tp[:, tt, :].rearrange("p (h d) -> p h d", h=2))
            return u

        groups["v"] = [vt_mm(0, 4, True, False), vt_mm(4, 8, False, True),
                       v_tr()]
        return groups

    def attn_units(g):
        """Per-kv-tile units: tile t's scores (both heads, concurrent PE row
        groups) + exp + mask, paired with tile t-1's AV matmuls (deferred so
        the PE never waits on the exp)."""
        b, j = divmod(g, 4)
        ntl = 4 * (j + 1)
        st = {}

        def sc_unit(t):
            def u():
                if t == 0:
                    if g not in avf:
                        avf[g] = sbp.tile([128, QSTRIP], BF16, tag="avf",
                                          name="avf")
                    st["av0"] = psV.tile([HD + 1, QSTRIP], F32, tag="av",
                                         name="av0_ps")
                    st["av1"] = psV.tile([HD + 1, QSTRIP], F32, tag="av",
                                         name="av1_ps")
                q0 = max(0, 128 * (t - 4 * j))
                sc_ps = psS.tile([128, 2, QSTRIP], F32, tag="sc", name="sc_ps")
                for h in range(2):
                    nc.tensor.matmul(
                        sc_ps[:, h, q0:],
                        lhsT=kt_sb[h * HD:(h + 1) * HD,
                                   b * S + t * 128: b * S + (t + 1) * 128],
                        rhs=qt_sb[h * HD:(h + 1) * HD,
                                  b * S + j * QSTRIP + q0:
                                  b * S + (j + 1) * QSTRIP],
                        start=True, stop=True)
                pexp = expp.tile([128, 2, QSTRIP], BF16, tag="pexp",
                                 name="pexp")
                nc.scalar.activation(
                    pexp[:, :, q0:], sc_ps[:, :, q0:],
                    mybir.ActivationFunctionType.Exp, scale=0.125)
                if t >= 4 * j:  # triangular mask on the diagonal 128-block
                    nc.vector.tensor_mul(
                        pexp[:, :, q0:q0 + 128], pexp[:, :, q0:q0 + 128],
                        msk_sb)
                st[f"pexp{t}"] = pexp
            return u

        def av_unit(t):
            def u():
                pexp = st.pop(f"pexp{t}")
                q0 = max(0, 128 * (t - 4 * j))
                for h in range(2):
                    idx = (b * HLOC + h) * KT_TILES + t
                    nc.tensor.matmul(
                        st[f"av{h}"][:, q0:], lhsT=v_sb[:, idx, :],
                        rhs=pexp[:, h, q0:],
                        start=(t == 0), stop=(t == ntl - 1))
            return u

        def norm_a():
            # evacuate both av accumulators from PSUM (freeing the banks for
            # the next strip) and shift the denominator rows to partition 0
            def u():
                av_sb = sbp.tile([HD + 1, 2 * QSTRIP], F32, tag="avs",
                                 name="av_sb")
                for h in range(2):
                    nc.vector.tensor_copy(
                        av_sb[:, h * QSTRIP:(h + 1) * QSTRIP], st[f"av{h}"])
                s0 = sbp.tile([1, 2 * QSTRIP], F32, tag="s0", name="s0")
                nc.gpsimd.dma_start(out=s0, in_=av_sb[HD:HD + 1, :])
                st["avs"] = av_sb
                st["s0"] = s0
            return u

        def norm_b():
            # broadcast the denominators across partitions on GpSimd in
            # per-head halves so the DVE recip/mul work pipelines behind the
            # first broadcast; head1 first so its avf shift DMA starts early
            def u():
                av_sb = st["avs"]
                rb = sbp.tile([HD, 2 * QSTRIP], F32, tag="rb")
                nc.gpsimd.partition_broadcast(rb[:, QSTRIP:],
                                              st["s0"][:, QSTRIP:],
                                              channels=HD)
                nc.gpsimd.partition_broadcast(rb[:, 0:QSTRIP],
                                              st["s0"][:, 0:QSTRIP],
                                              channels=HD)
                nc.vector.reciprocal_approx_fast(rb[:, QSTRIP:],
                                                 rb[:, QSTRIP:])
                avh = sbp.tile([HD, QSTRIP], BF16, tag="avh")
                nc.vector.tensor_mul(avh, av_sb[0:HD, QSTRIP:],
                                     rb[:, QSTRIP:])
                nc.gpsimd.dma_start(out=avf[g][HD:2 * HD, :], in_=avh)
                nc.vector.reciprocal_approx_fast(rb[:, 0:QSTRIP],
                                                 rb[:, 0:QSTRIP])
                nc.vector.tensor_mul(avf[g][0:HD, :],
                                     av_sb[0:HD, 0:QSTRIP],
                                     rb[:, 0:QSTRIP])
            return u

        # pair-batch kv tiles: [sc(t), sc(t+1)] then [av(t-2), av(t-1)] so
        # consecutive K=64 score matmuls avoid repeated ldweights stalls and
        # the AV matmuls trail one pair behind the exp that feeds them.
        units = []
        for t in range(ntl):
            units.append(sc_unit(t))
            if t % 2 == 1 and t > 1:
                units.append(av_unit(t - 3))
                units.append(av_unit(t - 2))
        units.append(av_unit(ntl - 2))
        units.append(av_unit(ntl - 1))
        units.append(norm_a())
        return units, norm_b()

    def out_units(g, copy_eng=None):
        gs = g * QSTRIP
        units = []

        def mk(tt):
            def u():
                ob = outp.tile([128, D], BF16, tag="ob")
                for n in range(2):
                    op_ps = psM.tile([128, 512], F32, tag="mm", name="op_ps")
                    nc.tensor.matmul(
                        op_ps, lhsT=avf[g][:, tt * 128:(tt + 1) * 128],
                        rhs=wo_sb[:, n * 512:(n + 1) * 512],
                        start=True, stop=True)
                    if copy_eng == "scalar":
                        nc.scalar.copy(ob[:, n * 512:(n + 1) * 512], op_ps)
                    else:
                        nc.vector.tensor_copy(ob[:, n * 512:(n + 1) * 512],
                                              op_ps)
                nc.sync.dma_start(
                    out=out[gs + tt * 128: gs + (tt + 1) * 128, :], in_=ob)
            return u
        for tt in range(4):
            units.append(mk(tt))
        return units

    # strip order: b0 ascending then b1 descending (short strips last).
    # fill[g] lists (strip, groups) of projection work + out strips whose
    # units are interleaved into attn(g)'s ACT-bound stretch.
    pu = {g: proj_units(g) for g in range(NSTRIP)}

    def pf(g, keys):
        return [u for k in keys for u in pu[g][k]]

    for u in pf(0, "qkv"):
        u()

    au = {}
    nb = {}
    for g in range(NSTRIP):
        au[g], nb[g] = attn_units(g)

    # norm_b(g) + out(g) ride in the NEXT strip's fill so the PE queue never
    # blocks on the norm chain at a strip boundary.
    fill_sched = {
        0: lambda: pf(1, "qkv"),
        1: lambda: pf(2, "qkv") + [nb[0]] + out_units(0),
        2: lambda: pf(3, "qkv") + pf(4, "kv") + [nb[1]] + out_units(1),
        3: lambda: pf(5, "kv") + pf(6, "kv") + pf(7, "qkv") + [nb[2]]
        + out_units(2),
        7: lambda: pf(6, "q") + [nb[3]] + out_units(3),
        6: lambda: pf(5, "q") + pf(4, "q") + [nb[7]] + out_units(7),
    }
    # dedicated (non-pooled) avT tensors for the tail strips 5 and 4:
    # no pool-slot reuse -> no WAR hazard across the tail window
    avf[5] = singles.tile([128, QSTRIP], BF16, tag="avf5", name="avf5")
    avf[4] = singles.tile([128, QSTRIP], BF16, tag="avf4", name="avf4")
    for g in [0, 1, 2, 3, 7, 6]:
        _interleave(au[g], fill_sched[g](), lead=0)
    _interleave(au[5], [nb[6]] + out_units(6), lead=0)
    _interleave(au[4], [nb[5]] + out_units(5), lead=0)
    nb[4]()
    for u in out_units(4):
        u()


_CACHED_NC = None


def build_module():
    global _CACHED_NC
    if _CACHED_NC is None:
        nc = bacc.Bacc("TRN2", debug=False)
        with tile.TileContext(nc) as tc:
            with ExitStack() as ctx:
                _build_kernel(ctx, tc)
        nc.compile()
        _CACHED_NC = nc
    return _CACHED_NC


def make_in_maps(x, Wq, Wk, Wv, Wo):
    x = np.asarray(x, np.float32)
    xT = x.reshape(NT, D).T.astype(NPBF16)          # [D, NT]
    # device layout [p, strip, k, col]: row d = k*128 + p
    xT = np.ascontiguousarray(
        xT.reshape(8, 128, NSTRIP, QSTRIP).transpose(1, 2, 0, 3))
    # diagonal-block causal mask, doubled over the 2 heads:
    # msk[i, h, c] = 1 if c >= i
    i = np.arange(128)[:, None, None]
    c = np.arange(128)[None, None, :]
    msk = np.broadcast_to((c >= i), (128, 2, 128)).astype(NPBF16)
    in_maps = []
    for core in range(NCORES):
        cs = slice(core * CW, (core + 1) * CW)
        def warr(W):  # [D, CW] -> [p, k, col] with d = k*128 + p
            a = np.asarray(W, np.float32)[:, cs].astype(NPBF16)
            return np.ascontiguousarray(
                a.reshape(8, 128, CW).transpose(1, 0, 2))
        in_maps.append({
            "xt": xT,
            "wq": warr(Wq),
            "wk": warr(Wk),
            "wv": warr(Wv),
            "wo": np.ascontiguousarray(np.asarray(Wo, np.float32)[cs, :]).astype(NPBF16),
            "msk": np.ascontiguousarray(msk),
            "idn": np.eye(128, dtype=NPBF16),
        })
    return in_maps


def kernel(x, Wq, bq, Wk, bk, Wv, bv, Wo, bo):
    for b_ in (bq, bk, bv, bo):
        assert np.count_nonzero(np.asarray(b_)) == 0, "nonzero biases unsupported"
    nc = build_module()
    in_maps = make_in_maps(x, Wq, Wk, Wv, Wo)
    res = run_bass_kernel_spmd(nc, in_maps, core_ids=list(range(NCORES)))
    partials = [res.results[c]["out"] for c in range(NCORES)]
    total = np.zeros((NT, D), np.float32)
    for p in partials:
        total += np.asarray(p, np.float32)
    return total.reshape(B, S, D)
